# revision 1
# baseline (speedup 1.0000x reference)
"""GAE (Generalized Advantage Estimation) Bass kernel for 8 Trainium2 cores.

Problem: rewards (2048, 8192) f32, values (2048, 8192) f32,
next_values (2048,) f32.
  next_v[:, t] = values[:, t+1] (t < S-1), next_values (t = S-1)
  deltas = rewards + GAMMA * next_v - values
  A_t = deltas_t + (GAMMA*LAM) * A_{t+1}   (A_S = 0, backward recurrence)
  advantages = A, returns = A + values

Sharding: pure data parallel over the batch dim — 2048 rows / 8 cores =
256 rows per core; the seq recurrence is row-local so there is no
cross-core communication.

Per core: 2 partition tiles of 128 rows x 8192 seq, processed as
uniform 2048-col chunks right-to-left. Returns satisfy their own
recurrence (B_t = [r_t + g(1-l)v_{t+1}] + gl*B_{t+1}, B_S = nv), which
needs one fewer elementwise pass than the advantages form; each chunk's
recurrence is a single DVE tensor_tensor_scan over a reversed
(negative-stride) view, chained right-to-left through the scan's
`initial` operand; advantages = returns - values. Everything stays in
the natural [batch, seq] layout so every DMA moves 8KB-contiguous
lines. next_values is loaded as one 512B row and spread across
partitions with a K=1 matmul (per-partition 4B DMAs would stall the
ring). Loads ride the sync HWDGE ring and stores the scalar ring:
direction-pure rings run fastest, and a store's data-ready wait
(handled at the issuing engine's sequencer) can never delay a load.
DVE is the pacer (~73us busy, scan = 2 cycles/elem, the two other
passes 1 cycle/elem); measured ~94.6us/core vs the ~90us structural
floor (load-ring fill + DVE busy + drain).
"""

import sys

if "/opt/trn_rl_repo" not in sys.path:
    sys.path.insert(0, "/opt/trn_rl_repo")

import numpy as np

GAMMA = 0.99
LAM = 0.95
C_COEF = GAMMA * LAM

B, S = 2048, 8192
N_CORES = 8
ROWS = B // N_CORES  # 256 rows per core
P = 128  # SBUF partitions
N_TILES = ROWS // P  # 2 row-tiles per core
# DMA granularity: every load/store moves a [128, 4096] block (16KB per
# partition line) to amortize per-instruction ring overhead. Compute
# sub-chunks inside each block ramp down at the global edges so the
# first scan starts early and the last scan is short. Right-to-left.
CHUNK = 2048

_CACHE: dict = {}


def _build():
    import concourse.bacc as bacc
    import concourse.mybir as mybir
    from concourse.tile import TileContext

    f32 = mybir.dt.float32
    add = mybir.AluOpType.add
    sub = mybir.AluOpType.subtract
    mult = mybir.AluOpType.mult

    nc = bacc.Bacc("TRN2", target_bir_lowering=False, name="gae8")
    r = nc.dram_tensor("rewards", [ROWS, S], f32, kind="ExternalInput")
    v = nc.dram_tensor("values", [ROWS, S], f32, kind="ExternalInput")
    nv = nc.dram_tensor("next_values", [ROWS], f32, kind="ExternalInput")
    adv = nc.dram_tensor("adv", [ROWS, S], f32, kind="ExternalOutput")
    ret = nc.dram_tensor("ret", [ROWS, S], f32, kind="ExternalOutput")

    # Returns satisfy their own backward recurrence, which needs one fewer
    # elementwise pass than the advantages form:
    #   B_t = e_t + c*B_{t+1},  e_t = r_t + gamma*(1-lam)*v_{t+1},  B_S = nv
    #   returns = B, advantages = B - v
    g1ml = GAMMA * (1.0 - LAM)

    with TileContext(nc) as tc:
        with (
            tc.tile_pool(name="cpool", bufs=1) as cpool,
            tc.tile_pool(name="psum", bufs=1, space="PSUM") as psum,
            tc.tile_pool(name="pool", bufs=5) as pool,
        ):
            c_t = cpool.tile([P, 1], f32)
            ones = cpool.tile([1, 1], f32)
            nvr = [
                cpool.tile([1, 128], f32, name=f"nvr{t}", tag=f"nvr{t}")
                for t in range(N_TILES)
            ]
            # next_values spread across partitions: a single 512B row load
            # (one DMA packet; per-partition 4B loads stall the ring), then
            # a K=1 matmul scatters it into a [128,1] PSUM column.
            nvp = [
                psum.tile([128, 1], f32, name=f"nvp{t}", tag=f"nvp{t}")
                for t in range(N_TILES)
            ]
            for t in range(N_TILES):
                nc.sync.dma_start(
                    out=nvr[t][:, :], in_=nv[t * P : (t + 1) * P].unsqueeze(0)
                )
            nc.vector.memset(c_t[:, :], C_COEF)
            nc.vector.memset(ones[:, :], 1.0)
            for t in range(N_TILES):
                nc.tensor.matmul(
                    nvp[t][:, :],
                    nvr[t][0:1, :],
                    ones[0:1, :],
                    start=True,
                    stop=True,
                )

            # Uniform 2048-col chunks, right-to-left per row-tile.
            # ALL loads on the sync HWDGE ring, ALL stores on the scalar
            # ring: direction-pure rings run fastest, and a store's
            # data-ready wait (handled at the issuing engine's sequencer)
            # can never delay a load.
            for t in range(N_TILES):
                rows = slice(t * P, (t + 1) * P)
                prev_ret = None
                edge_src = nvp[t][:, 0:1]
                col_end = S
                for ci in range(S // CHUNK):
                    W = CHUNK
                    col0 = col_end - W
                    last_chunk = t == N_TILES - 1 and ci == S // CHUNK - 1
                    # per-sub stores on the first chunk start the store
                    # ring ~9us earlier; on the last they shorten the tail
                    first_chunk = t == 0 and ci == 0
                    if last_chunk:
                        subs = [1536, 512]
                    elif first_chunk:
                        subs = [512, 1536]
                    else:
                        subs = [W]
                    v_t = pool.tile([P, W], f32)
                    r_t = pool.tile([P, W], f32)
                    ret_t = pool.tile([P, W], f32)
                    nc.sync.dma_start(out=v_t[:, :], in_=v[rows, col0 : col0 + W])
                    nc.sync.dma_start(out=r_t[:, :], in_=r[rows, col0 : col0 + W])

                    b = W
                    for w in subs:
                        a = b - w
                        # e = g1ml * v_next + r (in place over r_t). The
                        # chunk's rightmost column takes its successor from
                        # edge_src (the nv spread, or the previous chunk's
                        # first v column) via a 1-col split.
                        if b == W:
                            nc.vector.scalar_tensor_tensor(
                                out=r_t[:, a : W - 1],
                                in0=v_t[:, a + 1 : W],
                                scalar=g1ml,
                                in1=r_t[:, a : W - 1],
                                op0=mult,
                                op1=add,
                            )
                            nc.vector.scalar_tensor_tensor(
                                out=r_t[:, W - 1 : W],
                                in0=edge_src,
                                scalar=g1ml,
                                in1=r_t[:, W - 1 : W],
                                op0=mult,
                                op1=add,
                            )
                            init = (
                                nvp[t][:, 0:1]
                                if prev_ret is None
                                else prev_ret[:, 0:1]
                            )
                        else:
                            nc.vector.scalar_tensor_tensor(
                                out=r_t[:, a:b],
                                in0=v_t[:, a + 1 : b + 1],
                                scalar=g1ml,
                                in1=r_t[:, a:b],
                                op0=mult,
                                op1=add,
                            )
                            init = ret_t[:, b : b + 1]
                        # backward recurrence over reversed views:
                        # state = c*state + e -> returns
                        nc.vector.tensor_tensor_scan(
                            out=ret_t[:, a:b][:, ::-1],
                            data0=c_t[:, :].broadcast_to([P, w]),
                            data1=r_t[:, a:b][:, ::-1],
                            initial=init,
                            op0=mult,
                            op1=add,
                        )
                        # advantages = returns - v, into the freed e slots
                        nc.vector.tensor_tensor(
                            out=r_t[:, a:b],
                            in0=ret_t[:, a:b],
                            in1=v_t[:, a:b],
                            op=sub,
                        )
                        if last_chunk or first_chunk:
                            nc.scalar.dma_start(
                                out=ret[rows, col0 + a : col0 + b],
                                in_=ret_t[:, a:b],
                            )
                            nc.scalar.dma_start(
                                out=adv[rows, col0 + a : col0 + b],
                                in_=r_t[:, a:b],
                            )
                        b = a
                    if not (last_chunk or first_chunk):
                        nc.scalar.dma_start(
                            out=ret[rows, col0 : col0 + W], in_=ret_t[:, :]
                        )
                        nc.scalar.dma_start(
                            out=adv[rows, col0 : col0 + W], in_=r_t[:, :]
                        )
                    prev_ret = ret_t
                    edge_src = v_t[:, 0:1]
                    col_end = col0
    nc.finalize()
    return nc


def _get_nc():
    if "nc" not in _CACHE:
        _CACHE["nc"] = _build()
    return _CACHE["nc"]


def _run(rewards, values, next_values, **spmd_kwargs):
    """Shard over cores, run the Bass kernel, return BassKernelResults."""
    from concourse.bass_utils import run_bass_kernel_spmd

    nc = _get_nc()
    in_maps = []
    for c in range(N_CORES):
        sl = slice(c * ROWS, (c + 1) * ROWS)
        in_maps.append(
            {
                "rewards": np.ascontiguousarray(rewards[sl], dtype=np.float32),
                "values": np.ascontiguousarray(values[sl], dtype=np.float32),
                "next_values": np.ascontiguousarray(
                    next_values[sl], dtype=np.float32
                ),
            }
        )
    return run_bass_kernel_spmd(
        nc, in_maps, core_ids=list(range(N_CORES)), **spmd_kwargs
    )


def kernel(rewards, values, next_values):
    res = _run(rewards, values, next_values)
    advantages = np.concatenate([res.results[c]["adv"] for c in range(N_CORES)], 0)
    returns = np.concatenate([res.results[c]["ret"] for c in range(N_CORES)], 0)
    return advantages, returns

